# revision 13
# baseline (speedup 1.0000x reference)
import re
import sys
from contextlib import ExitStack

import numpy as np

try:
    import concourse  # noqa
except ImportError:
    sys.path.insert(0, "/opt/trn_rl_repo")

import ml_dtypes
import concourse.bass as bass  # noqa
import concourse.dve_ops as dve_ops
import concourse.tile as tile
from concourse import mybir
from concourse.bass_utils import run_bass_kernel_spmd
from concourse.dve_ops import DveOp
from concourse.dve_spec import C0, C1, Spec, Src0, Src1
from concourse.dve_table_gen import dve_ver_for
from concourse.bacc import Bacc

N_CORES = 8
B = 8192
BC = B // N_CORES  # 1024 batch per core
D_IN = 784
KT = 7  # 784 -> 7 k-tiles of 128
D_IN_PAD = KT * 128  # 896
D_H = 1000
HT = 8  # 1000 -> 8 h-tiles of 128
D_H_PAD = HT * 128  # 1024
D_OUT = 10
T = 25
BETA = 0.95
THR = 1.0
NTERMS = 3  # exact bf16 decomposition of W2
MW = 30  # packed fc2 stationary width: term ti at columns 10*ti..10*ti+9
GROUPS = [384, 384, 256]  # batch column groups; group g+1's fc1 drips through group g's loop
EVB = 3  # ev copies batched over EVB timesteps (PSUM rotation depth 2*EVB banks)
PACE_END = 17  # drip of next group's fc1 spread over steps 0..PACE_END-1
NSG = 2  # sgn buffer rotation depth
WMAX = 512

LAST_EXEC_NS = None
TRACE = False

_CACHE = {}


def _install_ntff_hook():
    try:
        import antenv.axon_hooks  # noqa

        return
    except ImportError:
        pass
    try:
        import types

        import antenv

        mod = types.ModuleType("antenv.axon_hooks")
        mod._hook = None

        def set_axon_ntff_profile_hook(h):
            mod._hook = h

        def get_axon_ntff_profile_hook():
            return mod._hook

        mod.set_axon_ntff_profile_hook = set_axon_ntff_profile_hook
        mod.get_axon_ntff_profile_hook = get_axon_ntff_profile_hook
        sys.modules["antenv.axon_hooks"] = mod
        antenv.axon_hooks = mod
        try:
            from trn_agent_boot.trn_boot import _ntff_profile_via_ctypes

            hook = _ntff_profile_via_ctypes("/opt/axon/libaxon_pjrt.so")
            if hook is not None:
                mod._hook = hook
        except Exception:
            pass
    except Exception:
        pass


def _register_memupd():
    for op in dve_ops.OPS:
        if op.name == "SNN_MEMUPD":
            return op
    spec = Spec(
        body=Src0 * C0 + Src1 - (Src0 > C1),
        reference=lambda in0, in1, s0, s1, imm2: in0 * s0
        + in1
        - (in0 > s1).astype(np.float32),
    )
    op = DveOp("SNN_MEMUPD", spec, subdim=False, uops_sha={})
    dve_ops.OPS.append(op)
    dve_ops.CUSTOM_DVE_SPECS[op.name] = op.spec
    dve_ops._SUB_OPCODE_FOR_NAME[op.name] = (
        dve_ops._CUSTOM_DVE_ROW_BASE + len(dve_ops.OPS) - 1
    )
    ver = dve_ver_for("TRN2")
    try:
        op.compile(ver)
    except ValueError as e:
        m = re.search(r'uops_sha\["%s"\]="([0-9a-f]+)"' % ver, str(e))
        if not m:
            raise
        op.uops_sha[ver] = m.group(1)
        op.compile(ver)
    return op


def _build():
    MEMUPD = _register_memupd()
    nc = Bacc()
    f32 = mybir.dt.float32
    bf16 = mybir.dt.bfloat16
    AF = mybir.ActivationFunctionType

    xT_d = nc.declare_dram_parameter("xT", [KT, 128, BC], f32, isOutput=False)
    w1T_d = nc.declare_dram_parameter("w1T", [KT, 128, D_H_PAD], f32, isOutput=False)
    b1r_d = nc.declare_dram_parameter("b1r", [128, HT], f32, isOutput=False)
    w2p_d = nc.declare_dram_parameter("w2p", [128, HT, MW], bf16, isOutput=False)
    evT_d = nc.declare_dram_parameter("evT", [T, MW, BC], f32, isOutput=True)

    col0 = [sum(GROUPS[:g]) for g in range(len(GROUPS))]  # start col of each group
    NG = len(GROUPS)

    with tile.TileContext(nc) as tc, ExitStack() as ctx:
        pool = ctx.enter_context(tc.tile_pool(name="sb", bufs=1))
        ppool = ctx.enter_context(tc.tile_pool(name="ps", bufs=1, space="PSUM"))

        xsb = pool.tile([128, KT, BC], f32)
        w1sb = pool.tile([128, KT, D_H_PAD], f32)
        cur1 = pool.tile([128, HT, BC], f32)
        mem = [pool.tile([128, HT, BC], f32, name=f"mem_{i}") for i in range(2)]
        sgn1 = [pool.tile([128, HT, BC], bf16, name=f"sgn1_{i}") for i in range(NSG)]
        w2sb = pool.tile([128, HT, MW], bf16)
        b1sb = pool.tile([128, HT], f32)
        negone = pool.tile([128, 1], f32)
        zero = pool.tile([128, 1], f32)
        ev = pool.tile([MW, EVB, WMAX], f32)

        p1 = [ppool.tile([128, WMAX], f32, name=f"p1_{i}") for i in range(2)]
        p2 = ppool.tile([MW, 2 * EVB, WMAX], f32)

        nc.gpsimd.memset(negone[:], -1.0)
        nc.gpsimd.memset(zero[:], 0.0)

        # DMA order tuned so PE can start group 0's fc1 early.
        nc.sync.dma_start(b1sb[:], b1r_d[:])
        nc.sync.dma_start(w2sb[:], w2p_d[:])
        g0 = slice(col0[0], col0[0] + GROUPS[0])
        for k in range(KT):
            nc.sync.dma_start(xsb[:, k, g0], xT_d[k, :, g0])
            nc.sync.dma_start(w1sb[:, k, 0:128], w1T_d[k, :, 0:128])
        for h in range(1, HT):
            for k in range(KT):
                nc.sync.dma_start(
                    w1sb[:, k, 128 * h : 128 * (h + 1)],
                    w1T_d[k, :, 128 * h : 128 * (h + 1)],
                )
        for g in range(1, NG):
            gs = slice(col0[g], col0[g] + GROUPS[g])
            for k in range(KT):
                nc.sync.dma_start(xsb[:, k, gs], xT_d[k, :, gs])

        def fc1_mm(g, h, k):
            bs = slice(col0[g], col0[g] + GROUPS[g])
            w = GROUPS[g]
            nc.tensor.matmul(
                p1[h % 2][:, 0:w],
                w1sb[:, k, 128 * h : 128 * (h + 1)],
                xsb[:, k, bs],
                start=(k == 0),
                stop=(k == KT - 1),
                skip_group_check=True,
            )

        def fc1_ident(g, h):
            bs = slice(col0[g], col0[g] + GROUPS[g])
            w = GROUPS[g]
            nc.scalar.activation(
                cur1[:, h, bs], p1[h % 2][:, 0:w], AF.Identity, bias=b1sb[:, h : h + 1]
            )

        # group 0's fc1 up front (the head)
        for h in range(HT):
            for k in range(KT):
                fc1_mm(0, h, k)
            fc1_ident(0, h)

        def drip_schedule():
            # spread next group's fc1 MMs over steps 0..PACE_END-1; identities
            # trail their h-tile by 2 steps so ACT never blocks on the PE.
            units = [(h, k) for h in range(HT) for k in range(KT)]
            U = len(units)
            mm_at = {}
            id_at = {}
            for u, (h, k) in enumerate(units):
                s = u * PACE_END // U
                mm_at.setdefault(s, []).append((h, k))
                if k == KT - 1:
                    id_at.setdefault(s + 2, []).append(h)
            return mm_at, id_at

        # Flat emission over (group, t) slots on a virtual timeline: group
        # g+1's first steps are emitted OV steps before group g's last step,
        # so no engine idles across the group boundary. PSUM banks and sgn
        # buffers rotate on the GLOBAL step index so overlapped slots never
        # collide.
        OV = 3
        slots = []
        for g in range(NG):
            for t in range(T):
                slots.append((g * (T - OV) + t, g, t))
        slots.sort(key=lambda x: (x[0], x[1]))

        drips = [drip_schedule() if g + 1 < NG else ({}, {}) for g in range(NG)]
        pending_copies = []  # (g, ts) ev batches, emitted one slot late

        def fc2_step(g, t):
            bs = slice(col0[g], col0[g] + GROUPS[g])
            w = GROUPS[g]
            gs = g * T + t
            for h in range(HT):
                nc.tensor.matmul(
                    p2[:, gs % (2 * EVB), 0:w],
                    w2sb[:, h, :],
                    sgn1[gs % NSG][:, h, bs],
                    start=(h == 0),
                    stop=(h == HT - 1),
                    skip_group_check=True,
                )

        def ev_out(g, ts):
            bs = slice(col0[g], col0[g] + GROUPS[g])
            w = GROUPS[g]
            nb = len(ts)
            j0 = (g * T + ts[0]) % (2 * EVB)
            # copy PSUM banks j0..j0+nb-1 (mod 6) into ev[0:nb]; the bank
            # window can wrap, needing two activations
            n1 = min(nb, 2 * EVB - j0)
            nc.scalar.activation(
                ev[:, 0:n1, 0:w],
                p2[:, j0 : j0 + n1, 0:w],
                AF.Identity,
                bias=zero[0:MW],
            )
            if n1 < nb:
                nc.scalar.activation(
                    ev[:, n1:nb, 0:w],
                    p2[:, 0 : nb - n1, 0:w],
                    AF.Identity,
                    bias=zero[0:MW],
                )
            for i, tt in enumerate(ts):
                nc.sync.dma_start(evT_d[tt, :, bs], ev[:, i, 0:w])

        for _, g, t in slots:
            bs = slice(col0[g], col0[g] + GROUPS[g])
            w = GROUPS[g]
            gs = g * T + t
            # DVE: membrane update (ping-pong buffers, no WAR with sign).
            # Group 0's first update is split by h-halves so it can start
            # while the head fc1 is still producing later h-tiles.
            if t == 1 and g == 0:
                for hh in range(2):
                    hs = slice(4 * hh, 4 * hh + 4)
                    nc.vector._custom_dve(
                        MEMUPD,
                        out=mem[1][:, hs, bs],
                        in0=cur1[:, hs, bs],
                        in1=cur1[:, hs, bs],
                        s0=BETA,
                        s1=THR,
                    )
            elif t >= 1:
                src = cur1 if t == 1 else mem[(t - 1) % 2]
                nc.vector._custom_dve(
                    MEMUPD,
                    out=mem[t % 2][:, :, bs],
                    in0=src[:, :, bs],
                    in1=cur1[:, :, bs],
                    s0=BETA,
                    s1=THR,
                )
            # ACT: spike sign (group 0's t=0 split by halves to start earlier)
            msrc = cur1 if t == 0 else mem[t % 2]
            if t == 0 and g == 0:
                for hh in range(2):
                    hs = slice(4 * hh, 4 * hh + 4)
                    nc.scalar.activation(
                        sgn1[gs % NSG][:, hs, bs],
                        msrc[:, hs, bs],
                        AF.Sign,
                        bias=negone[:],
                    )
            else:
                nc.scalar.activation(
                    sgn1[gs % NSG][:, :, bs],
                    msrc[:, :, bs],
                    AF.Sign,
                    bias=negone[:],
                )
            # PE: drip next group's fc1 (before fc2 so it runs while fc2
            # waits on this step's sign). Identities are emitted before new
            # MMs: fc1(h+2) reuses p1[h%2], so ident(h) must read it first.
            mm_at, id_at = drips[g]
            for h in id_at.get(t, []):
                fc1_ident(g + 1, h)
            for h, k in mm_at.get(t, []):
                fc1_mm(g + 1, h, k)
            # PE: fc2 for this step
            fc2_step(g, t)
            # ACT: ev copy batches, one slot after their last fc2
            for pg, pts in pending_copies:
                ev_out(pg, pts)
            pending_copies.clear()
            if t % EVB == EVB - 1:
                pending_copies.append((g, list(range(t - EVB + 1, t + 1))))
            elif t == T - 1:
                pending_copies.append((g, list(range(T - T % EVB, T))))
        for pg, pts in pending_copies:
            ev_out(pg, pts)

    nc.finalize()
    return nc


def _prep_shared(W1, b1, W2, b2):
    bf = ml_dtypes.bfloat16
    w1T = np.zeros((KT * 128, D_H_PAD), np.float32)
    w1T[:D_IN, :D_H] = W1.T
    w1T = np.ascontiguousarray(w1T.reshape(KT, 128, D_H_PAD))

    b1pad = np.zeros(D_H_PAD, np.float32)
    b1pad[:D_H] = b1
    b1r = np.ascontiguousarray(b1pad.reshape(HT, 128).T)

    w2pad = np.zeros((D_OUT, D_H_PAD), np.float32)
    w2pad[:, :D_H] = W2
    terms = []
    r = w2pad.copy()
    for _ in range(NTERMS):
        tb = r.astype(bf)
        terms.append(tb)
        r = r - tb.astype(np.float32)

    w2p = np.zeros((128, HT, MW), bf)
    for h in range(HT):
        for ti, tb in enumerate(terms):
            half = (0.5 * tb[:, 128 * h : 128 * (h + 1)].astype(np.float32)).astype(bf)
            w2p[:, h, D_OUT * ti : D_OUT * (ti + 1)] = half.T

    # spk = (sgn+1)/2 so W2@spk = 0.5*W2@sgn + 0.5*sum(W2); fold shift into v.
    v = (b2.astype(np.float64) + 0.5 * w2pad.astype(np.float64).sum(axis=1)).astype(
        np.float32
    )
    return w1T, b1r, w2p, v


def kernel(**inputs):
    global LAST_EXEC_NS
    x = np.ascontiguousarray(np.asarray(inputs["x"], dtype=np.float32))
    W1 = np.asarray(inputs["W1"], dtype=np.float32)
    b1 = np.asarray(inputs["b1"], dtype=np.float32)
    W2 = np.asarray(inputs["W2"], dtype=np.float32)
    b2 = np.asarray(inputs["b2"], dtype=np.float32)

    if "nc" not in _CACHE:
        _CACHE["nc"] = _build()
    nc = _CACHE["nc"]

    w1T, b1r, w2p, v = _prep_shared(W1, b1, W2, b2)

    in_maps = []
    for c in range(N_CORES):
        xc = x[c * BC : (c + 1) * BC]  # [BC, 784]
        xT = np.zeros((KT * 128, BC), np.float32)
        xT[:D_IN] = xc.T
        in_maps.append(
            {
                "xT": np.ascontiguousarray(xT.reshape(KT, 128, BC)),
                "w1T": w1T,
                "b1r": b1r,
                "w2p": w2p,
            }
        )

    if TRACE:
        _install_ntff_hook()
    br = run_bass_kernel_spmd(nc, in_maps, list(range(N_CORES)), trace=TRACE)
    LAST_EXEC_NS = br.exec_time_ns

    # cur2 = 0.5*W2@sgn + v, summed over the 3 bf16 terms on host.
    X = np.empty((T, B, D_OUT), np.float32)
    for c in range(N_CORES):
        evT = br.results[c]["evT"]  # [T, MW, BC]
        s = (evT[:, 0:D_OUT, :] + evT[:, D_OUT : 2 * D_OUT, :]) + evT[
            :, 2 * D_OUT : 3 * D_OUT, :
        ]
        X[:, c * BC : (c + 1) * BC, :] = np.transpose(s, (0, 2, 1)) + v

    mem2_rec = np.empty((T, B, D_OUT), np.float32)
    beta = np.float32(BETA)
    mem = X[0]
    mem2_rec[0] = mem
    for t in range(1, T):
        spk = (mem > THR).astype(np.float32)
        mem = beta * mem + X[t] - spk
        mem2_rec[t] = mem
    spk2_rec = (mem2_rec > THR).astype(np.float32)
    return spk2_rec, mem2_rec


# revision 14
# speedup vs baseline: 1.0291x; 1.0291x over previous
import re
import sys
from contextlib import ExitStack

import numpy as np

try:
    import concourse  # noqa
except ImportError:
    sys.path.insert(0, "/opt/trn_rl_repo")

import ml_dtypes
import concourse.bass as bass  # noqa
import concourse.dve_ops as dve_ops
import concourse.tile as tile
from concourse import mybir
from concourse.bass_utils import run_bass_kernel_spmd
from concourse.dve_ops import DveOp
from concourse.dve_spec import C0, C1, Spec, Src0, Src1
from concourse.dve_table_gen import dve_ver_for
from concourse.bacc import Bacc

N_CORES = 8
B = 8192
BC = B // N_CORES  # 1024 batch per core
D_IN = 784
KT = 7  # 784 -> 7 k-tiles of 128
D_IN_PAD = KT * 128  # 896
D_H = 1000
HT = 8  # 1000 -> 8 h-tiles of 128
D_H_PAD = HT * 128  # 1024
D_OUT = 10
T = 25
BETA = 0.95
THR = 1.0
NTERMS = 3  # exact bf16 decomposition of W2
MW = 30  # packed fc2 stationary width: term ti at columns 10*ti..10*ti+9
GROUPS = [512, 512]  # batch column groups; group g+1's fc1 drips through group g's loop
EVB = 3  # ev copies batched over EVB timesteps (PSUM rotation depth 2*EVB banks)
PACE_END = 17  # drip of next group's fc1 spread over steps 0..PACE_END-1
NSG = 3  # sgn buffer rotation depth
WMAX = 512

LAST_EXEC_NS = None
TRACE = False

_CACHE = {}


def _install_ntff_hook():
    try:
        import antenv.axon_hooks  # noqa

        return
    except ImportError:
        pass
    try:
        import types

        import antenv

        mod = types.ModuleType("antenv.axon_hooks")
        mod._hook = None

        def set_axon_ntff_profile_hook(h):
            mod._hook = h

        def get_axon_ntff_profile_hook():
            return mod._hook

        mod.set_axon_ntff_profile_hook = set_axon_ntff_profile_hook
        mod.get_axon_ntff_profile_hook = get_axon_ntff_profile_hook
        sys.modules["antenv.axon_hooks"] = mod
        antenv.axon_hooks = mod
        try:
            from trn_agent_boot.trn_boot import _ntff_profile_via_ctypes

            hook = _ntff_profile_via_ctypes("/opt/axon/libaxon_pjrt.so")
            if hook is not None:
                mod._hook = hook
        except Exception:
            pass
    except Exception:
        pass


def _register_memupd():
    for op in dve_ops.OPS:
        if op.name == "SNN_MEMUPD":
            return op
    spec = Spec(
        body=Src0 * C0 + Src1 - (Src0 > C1),
        reference=lambda in0, in1, s0, s1, imm2: in0 * s0
        + in1
        - (in0 > s1).astype(np.float32),
    )
    op = DveOp("SNN_MEMUPD", spec, subdim=False, uops_sha={})
    dve_ops.OPS.append(op)
    dve_ops.CUSTOM_DVE_SPECS[op.name] = op.spec
    dve_ops._SUB_OPCODE_FOR_NAME[op.name] = (
        dve_ops._CUSTOM_DVE_ROW_BASE + len(dve_ops.OPS) - 1
    )
    ver = dve_ver_for("TRN2")
    try:
        op.compile(ver)
    except ValueError as e:
        m = re.search(r'uops_sha\["%s"\]="([0-9a-f]+)"' % ver, str(e))
        if not m:
            raise
        op.uops_sha[ver] = m.group(1)
        op.compile(ver)
    return op


def _build():
    MEMUPD = _register_memupd()
    nc = Bacc()
    f32 = mybir.dt.float32
    bf16 = mybir.dt.bfloat16
    AF = mybir.ActivationFunctionType

    xT_d = nc.declare_dram_parameter("xT", [KT, 128, BC], f32, isOutput=False)
    w1T_d = nc.declare_dram_parameter("w1T", [KT, 128, D_H_PAD], f32, isOutput=False)
    b1r_d = nc.declare_dram_parameter("b1r", [128, HT], f32, isOutput=False)
    w2p_d = nc.declare_dram_parameter("w2p", [128, HT, MW], bf16, isOutput=False)
    evT_d = nc.declare_dram_parameter("evT", [T, MW, BC], f32, isOutput=True)

    col0 = [sum(GROUPS[:g]) for g in range(len(GROUPS))]  # start col of each group
    NG = len(GROUPS)

    with tile.TileContext(nc) as tc, ExitStack() as ctx:
        pool = ctx.enter_context(tc.tile_pool(name="sb", bufs=1))
        ppool = ctx.enter_context(tc.tile_pool(name="ps", bufs=1, space="PSUM"))

        xsb = pool.tile([128, KT, BC], f32)
        w1sb = pool.tile([128, KT, D_H_PAD], f32)
        cur1 = pool.tile([128, HT, BC], f32)
        mem = [pool.tile([128, HT, BC], f32, name=f"mem_{i}") for i in range(2)]
        sgn1 = [pool.tile([128, HT, BC], bf16, name=f"sgn1_{i}") for i in range(NSG)]
        w2sb = pool.tile([128, HT, MW], bf16)
        b1sb = pool.tile([128, HT], f32)
        negone = pool.tile([128, 1], f32)
        zero = pool.tile([128, 1], f32)
        ev = pool.tile([MW, EVB, WMAX], f32)

        p1 = [ppool.tile([128, WMAX], f32, name=f"p1_{i}") for i in range(2)]
        p2 = ppool.tile([MW, 2 * EVB, WMAX], f32)

        nc.gpsimd.memset(negone[:], -1.0)
        nc.gpsimd.memset(zero[:], 0.0)

        # DMA order tuned so PE can start group 0's fc1 early.
        nc.sync.dma_start(b1sb[:], b1r_d[:])
        nc.sync.dma_start(w2sb[:], w2p_d[:])
        g0 = slice(col0[0], col0[0] + GROUPS[0])
        for k in range(KT):
            nc.sync.dma_start(xsb[:, k, g0], xT_d[k, :, g0])
            nc.sync.dma_start(w1sb[:, k, 0:128], w1T_d[k, :, 0:128])
        for h in range(1, HT):
            for k in range(KT):
                nc.sync.dma_start(
                    w1sb[:, k, 128 * h : 128 * (h + 1)],
                    w1T_d[k, :, 128 * h : 128 * (h + 1)],
                )
        for g in range(1, NG):
            gs = slice(col0[g], col0[g] + GROUPS[g])
            for k in range(KT):
                nc.sync.dma_start(xsb[:, k, gs], xT_d[k, :, gs])

        def fc1_mm(g, h, k):
            bs = slice(col0[g], col0[g] + GROUPS[g])
            w = GROUPS[g]
            nc.tensor.matmul(
                p1[h % 2][:, 0:w],
                w1sb[:, k, 128 * h : 128 * (h + 1)],
                xsb[:, k, bs],
                start=(k == 0),
                stop=(k == KT - 1),
                skip_group_check=True,
            )

        def fc1_ident(g, h):
            bs = slice(col0[g], col0[g] + GROUPS[g])
            w = GROUPS[g]
            nc.scalar.activation(
                cur1[:, h, bs], p1[h % 2][:, 0:w], AF.Identity, bias=b1sb[:, h : h + 1]
            )

        # group 0's fc1 up front (the head)
        for h in range(HT):
            for k in range(KT):
                fc1_mm(0, h, k)
            fc1_ident(0, h)

        def drip_schedule():
            # spread next group's fc1 MMs over steps 0..PACE_END-1; identities
            # trail their h-tile by 2 steps so ACT never blocks on the PE.
            units = [(h, k) for h in range(HT) for k in range(KT)]
            U = len(units)
            mm_at = {}
            id_at = {}
            for u, (h, k) in enumerate(units):
                s = u * PACE_END // U
                mm_at.setdefault(s, []).append((h, k))
                if k == KT - 1:
                    id_at.setdefault(s + 2, []).append(h)
            return mm_at, id_at

        # Flat emission over (group, t) slots on a virtual timeline: group
        # g+1's first steps are emitted OV steps before group g's last step,
        # so no engine idles across the group boundary. PSUM banks and sgn
        # buffers rotate on the GLOBAL step index so overlapped slots never
        # collide.
        OV = 3
        slots = []
        for g in range(NG):
            for t in range(T):
                slots.append((g * (T - OV) + t, g, t))
        slots.sort(key=lambda x: (x[0], x[1]))

        drips = [drip_schedule() if g + 1 < NG else ({}, {}) for g in range(NG)]
        pending_copies = []  # (g, ts) ev batches, emitted one slot late

        def fc2_step(g, t):
            bs = slice(col0[g], col0[g] + GROUPS[g])
            w = GROUPS[g]
            gs = g * T + t
            for h in range(HT):
                nc.tensor.matmul(
                    p2[:, gs % (2 * EVB), 0:w],
                    w2sb[:, h, :],
                    sgn1[gs % NSG][:, h, bs],
                    start=(h == 0),
                    stop=(h == HT - 1),
                    skip_group_check=True,
                )

        def ev_out(g, ts):
            bs = slice(col0[g], col0[g] + GROUPS[g])
            w = GROUPS[g]
            nb = len(ts)
            j0 = (g * T + ts[0]) % (2 * EVB)
            # copy PSUM banks j0..j0+nb-1 (mod 6) into ev[0:nb]; the bank
            # window can wrap, needing two activations
            n1 = min(nb, 2 * EVB - j0)
            nc.scalar.activation(
                ev[:, 0:n1, 0:w],
                p2[:, j0 : j0 + n1, 0:w],
                AF.Identity,
                bias=zero[0:MW],
            )
            if n1 < nb:
                nc.scalar.activation(
                    ev[:, n1:nb, 0:w],
                    p2[:, 0 : nb - n1, 0:w],
                    AF.Identity,
                    bias=zero[0:MW],
                )
            for i, tt in enumerate(ts):
                nc.sync.dma_start(evT_d[tt, :, bs], ev[:, i, 0:w])

        for _, g, t in slots:
            bs = slice(col0[g], col0[g] + GROUPS[g])
            w = GROUPS[g]
            gs = g * T + t
            # DVE: membrane update (ping-pong buffers, no WAR with sign).
            # Group 0's first update is split by h-halves so it can start
            # while the head fc1 is still producing later h-tiles.
            if t == 1 and g == 0:
                for hh in range(2):
                    hs = slice(4 * hh, 4 * hh + 4)
                    nc.vector._custom_dve(
                        MEMUPD,
                        out=mem[1][:, hs, bs],
                        in0=cur1[:, hs, bs],
                        in1=cur1[:, hs, bs],
                        s0=BETA,
                        s1=THR,
                    )
            elif t >= 1:
                src = cur1 if t == 1 else mem[(t - 1) % 2]
                nc.vector._custom_dve(
                    MEMUPD,
                    out=mem[t % 2][:, :, bs],
                    in0=src[:, :, bs],
                    in1=cur1[:, :, bs],
                    s0=BETA,
                    s1=THR,
                )
            # ACT: spike sign (group 0's t=0 split by halves to start earlier)
            msrc = cur1 if t == 0 else mem[t % 2]
            if t == 0 and g == 0:
                for hh in range(2):
                    hs = slice(4 * hh, 4 * hh + 4)
                    nc.scalar.activation(
                        sgn1[gs % NSG][:, hs, bs],
                        msrc[:, hs, bs],
                        AF.Sign,
                        bias=negone[:],
                    )
            else:
                nc.scalar.activation(
                    sgn1[gs % NSG][:, :, bs],
                    msrc[:, :, bs],
                    AF.Sign,
                    bias=negone[:],
                )
            # PE: drip next group's fc1 (before fc2 so it runs while fc2
            # waits on this step's sign). Identities are emitted before new
            # MMs: fc1(h+2) reuses p1[h%2], so ident(h) must read it first.
            mm_at, id_at = drips[g]
            for h in id_at.get(t, []):
                fc1_ident(g + 1, h)
            for h, k in mm_at.get(t, []):
                fc1_mm(g + 1, h, k)
            # PE: fc2 for this step
            fc2_step(g, t)
            # ACT: ev copy batches, one slot after their last fc2
            for pg, pts in pending_copies:
                ev_out(pg, pts)
            pending_copies.clear()
            if t % EVB == EVB - 1:
                pending_copies.append((g, list(range(t - EVB + 1, t + 1))))
            elif t == T - 1:
                pending_copies.append((g, list(range(T - T % EVB, T))))
        for pg, pts in pending_copies:
            ev_out(pg, pts)

    nc.finalize()
    return nc


def _prep_shared(W1, b1, W2, b2):
    bf = ml_dtypes.bfloat16
    w1T = np.zeros((KT * 128, D_H_PAD), np.float32)
    w1T[:D_IN, :D_H] = W1.T
    w1T = np.ascontiguousarray(w1T.reshape(KT, 128, D_H_PAD))

    b1pad = np.zeros(D_H_PAD, np.float32)
    b1pad[:D_H] = b1
    b1r = np.ascontiguousarray(b1pad.reshape(HT, 128).T)

    w2pad = np.zeros((D_OUT, D_H_PAD), np.float32)
    w2pad[:, :D_H] = W2
    terms = []
    r = w2pad.copy()
    for _ in range(NTERMS):
        tb = r.astype(bf)
        terms.append(tb)
        r = r - tb.astype(np.float32)

    w2p = np.zeros((128, HT, MW), bf)
    for h in range(HT):
        for ti, tb in enumerate(terms):
            half = (0.5 * tb[:, 128 * h : 128 * (h + 1)].astype(np.float32)).astype(bf)
            w2p[:, h, D_OUT * ti : D_OUT * (ti + 1)] = half.T

    # spk = (sgn+1)/2 so W2@spk = 0.5*W2@sgn + 0.5*sum(W2); fold shift into v.
    v = (b2.astype(np.float64) + 0.5 * w2pad.astype(np.float64).sum(axis=1)).astype(
        np.float32
    )
    return w1T, b1r, w2p, v


def kernel(**inputs):
    global LAST_EXEC_NS
    x = np.ascontiguousarray(np.asarray(inputs["x"], dtype=np.float32))
    W1 = np.asarray(inputs["W1"], dtype=np.float32)
    b1 = np.asarray(inputs["b1"], dtype=np.float32)
    W2 = np.asarray(inputs["W2"], dtype=np.float32)
    b2 = np.asarray(inputs["b2"], dtype=np.float32)

    if "nc" not in _CACHE:
        _CACHE["nc"] = _build()
    nc = _CACHE["nc"]

    w1T, b1r, w2p, v = _prep_shared(W1, b1, W2, b2)

    in_maps = []
    for c in range(N_CORES):
        xc = x[c * BC : (c + 1) * BC]  # [BC, 784]
        xT = np.zeros((KT * 128, BC), np.float32)
        xT[:D_IN] = xc.T
        in_maps.append(
            {
                "xT": np.ascontiguousarray(xT.reshape(KT, 128, BC)),
                "w1T": w1T,
                "b1r": b1r,
                "w2p": w2p,
            }
        )

    if TRACE:
        _install_ntff_hook()
    br = run_bass_kernel_spmd(nc, in_maps, list(range(N_CORES)), trace=TRACE)
    LAST_EXEC_NS = br.exec_time_ns

    # cur2 = 0.5*W2@sgn + v, summed over the 3 bf16 terms on host.
    X = np.empty((T, B, D_OUT), np.float32)
    for c in range(N_CORES):
        evT = br.results[c]["evT"]  # [T, MW, BC]
        s = (evT[:, 0:D_OUT, :] + evT[:, D_OUT : 2 * D_OUT, :]) + evT[
            :, 2 * D_OUT : 3 * D_OUT, :
        ]
        X[:, c * BC : (c + 1) * BC, :] = np.transpose(s, (0, 2, 1)) + v

    mem2_rec = np.empty((T, B, D_OUT), np.float32)
    beta = np.float32(BETA)
    mem = X[0]
    mem2_rec[0] = mem
    for t in range(1, T):
        spk = (mem > THR).astype(np.float32)
        mem = beta * mem + X[t] - spk
        mem2_rec[t] = mem
    spk2_rec = (mem2_rec > THR).astype(np.float32)
    return spk2_rec, mem2_rec


# revision 16
# speedup vs baseline: 1.0372x; 1.0079x over previous
import re
import sys
from contextlib import ExitStack

import numpy as np

try:
    import concourse  # noqa
except ImportError:
    sys.path.insert(0, "/opt/trn_rl_repo")

import ml_dtypes
import concourse.bass as bass  # noqa
import concourse.dve_ops as dve_ops
import concourse.tile as tile
from concourse import mybir
from concourse.bass_utils import run_bass_kernel_spmd
from concourse.dve_ops import DveOp
from concourse.dve_spec import C0, C1, Spec, Src0, Src1
from concourse.dve_table_gen import dve_ver_for
from concourse.bacc import Bacc

N_CORES = 8
B = 8192
BC = B // N_CORES  # 1024 batch per core
D_IN = 784
KT = 7  # 784 -> 7 k-tiles of 128
D_IN_PAD = KT * 128  # 896
D_H = 1000
HT = 8  # 1000 -> 8 h-tiles of 128
D_H_PAD = HT * 128  # 1024
D_OUT = 10
T = 25
BETA = 0.95
THR = 1.0
NTERMS = 3  # exact bf16 decomposition of W2
MW = 30  # packed fc2 stationary width: term ti at columns 10*ti..10*ti+9
GROUPS = [512, 512]  # batch column groups; group g+1's fc1 drips through group g's loop
EVB = 3  # ev copies batched over EVB timesteps (PSUM rotation depth 2*EVB banks)
PACE_END = 17  # drip of next group's fc1 spread over steps 0..PACE_END-1
NSG = 3  # sgn buffer rotation depth
WMAX = 512

LAST_EXEC_NS = None
TRACE = False

_CACHE = {}


def _install_ntff_hook():
    try:
        import antenv.axon_hooks  # noqa

        return
    except ImportError:
        pass
    try:
        import types

        import antenv

        mod = types.ModuleType("antenv.axon_hooks")
        mod._hook = None

        def set_axon_ntff_profile_hook(h):
            mod._hook = h

        def get_axon_ntff_profile_hook():
            return mod._hook

        mod.set_axon_ntff_profile_hook = set_axon_ntff_profile_hook
        mod.get_axon_ntff_profile_hook = get_axon_ntff_profile_hook
        sys.modules["antenv.axon_hooks"] = mod
        antenv.axon_hooks = mod
        try:
            from trn_agent_boot.trn_boot import _ntff_profile_via_ctypes

            hook = _ntff_profile_via_ctypes("/opt/axon/libaxon_pjrt.so")
            if hook is not None:
                mod._hook = hook
        except Exception:
            pass
    except Exception:
        pass


def _register_memupd():
    for op in dve_ops.OPS:
        if op.name == "SNN_MEMUPD":
            return op
    spec = Spec(
        body=Src0 * C0 + Src1 - (Src0 > C1),
        reference=lambda in0, in1, s0, s1, imm2: in0 * s0
        + in1
        - (in0 > s1).astype(np.float32),
    )
    op = DveOp("SNN_MEMUPD", spec, subdim=False, uops_sha={})
    dve_ops.OPS.append(op)
    dve_ops.CUSTOM_DVE_SPECS[op.name] = op.spec
    dve_ops._SUB_OPCODE_FOR_NAME[op.name] = (
        dve_ops._CUSTOM_DVE_ROW_BASE + len(dve_ops.OPS) - 1
    )
    ver = dve_ver_for("TRN2")
    try:
        op.compile(ver)
    except ValueError as e:
        m = re.search(r'uops_sha\["%s"\]="([0-9a-f]+)"' % ver, str(e))
        if not m:
            raise
        op.uops_sha[ver] = m.group(1)
        op.compile(ver)
    return op


def _build():
    MEMUPD = _register_memupd()
    nc = Bacc()
    f32 = mybir.dt.float32
    bf16 = mybir.dt.bfloat16
    AF = mybir.ActivationFunctionType

    xT_d = nc.declare_dram_parameter("xT", [KT, 128, BC], f32, isOutput=False)
    w1T_d = nc.declare_dram_parameter("w1T", [KT, 128, D_H_PAD], f32, isOutput=False)
    b1r_d = nc.declare_dram_parameter("b1r", [128, HT], f32, isOutput=False)
    w2p_d = nc.declare_dram_parameter("w2p", [128, HT, MW], bf16, isOutput=False)
    evT_d = nc.declare_dram_parameter("evT", [T, MW, BC], f32, isOutput=True)

    col0 = [sum(GROUPS[:g]) for g in range(len(GROUPS))]  # start col of each group
    NG = len(GROUPS)

    with tile.TileContext(nc) as tc, ExitStack() as ctx:
        pool = ctx.enter_context(tc.tile_pool(name="sb", bufs=1))
        ppool = ctx.enter_context(tc.tile_pool(name="ps", bufs=1, space="PSUM"))

        xsb = pool.tile([128, KT, BC], f32)
        w1sb = pool.tile([128, KT, D_H_PAD], f32)
        cur1 = pool.tile([128, HT, BC], f32)
        mem = [pool.tile([128, HT, BC], f32, name=f"mem_{i}") for i in range(2)]
        sgn1 = [pool.tile([128, HT, BC], bf16, name=f"sgn1_{i}") for i in range(NSG)]
        w2sb = pool.tile([128, HT, MW], bf16)
        b1sb = pool.tile([128, HT], f32)
        negone = pool.tile([128, 1], f32)
        zero = pool.tile([128, 1], f32)
        ev = pool.tile([MW, EVB, WMAX], f32)

        p1 = [ppool.tile([128, WMAX], f32, name=f"p1_{i}") for i in range(2)]
        p2 = ppool.tile([MW, 2 * EVB, WMAX], f32)

        nc.gpsimd.memset(negone[:], -1.0)
        nc.gpsimd.memset(zero[:], 0.0)

        # DMA order tuned so PE can start group 0's fc1 early.
        nc.sync.dma_start(b1sb[:], b1r_d[:])
        nc.sync.dma_start(w2sb[:], w2p_d[:])
        g0 = slice(col0[0], col0[0] + GROUPS[0])
        for k in range(KT):
            nc.sync.dma_start(xsb[:, k, g0], xT_d[k, :, g0])
            nc.sync.dma_start(w1sb[:, k, 0:128], w1T_d[k, :, 0:128])
        for h in range(1, HT):
            for k in range(KT):
                nc.sync.dma_start(
                    w1sb[:, k, 128 * h : 128 * (h + 1)],
                    w1T_d[k, :, 128 * h : 128 * (h + 1)],
                )
        for g in range(1, NG):
            gs = slice(col0[g], col0[g] + GROUPS[g])
            for k in range(KT):
                nc.sync.dma_start(xsb[:, k, gs], xT_d[k, :, gs])

        def fc1_mm(g, h, k):
            bs = slice(col0[g], col0[g] + GROUPS[g])
            w = GROUPS[g]
            nc.tensor.matmul(
                p1[h % 2][:, 0:w],
                w1sb[:, k, 128 * h : 128 * (h + 1)],
                xsb[:, k, bs],
                start=(k == 0),
                stop=(k == KT - 1),
                skip_group_check=True,
            )

        def fc1_ident(g, h):
            bs = slice(col0[g], col0[g] + GROUPS[g])
            w = GROUPS[g]
            nc.scalar.activation(
                cur1[:, h, bs], p1[h % 2][:, 0:w], AF.Identity, bias=b1sb[:, h : h + 1]
            )

        # group 0's fc1 up front (the head)
        for h in range(HT):
            for k in range(KT):
                fc1_mm(0, h, k)
            fc1_ident(0, h)

        def drip_schedule():
            # spread next group's fc1 MMs over steps 0..PACE_END-1; identities
            # trail their h-tile by 2 steps so ACT never blocks on the PE.
            units = [(h, k) for h in range(HT) for k in range(KT)]
            U = len(units)
            mm_at = {}
            id_at = {}
            for u, (h, k) in enumerate(units):
                s = u * PACE_END // U
                mm_at.setdefault(s, []).append((h, k))
                if k == KT - 1:
                    id_at.setdefault(s + 2, []).append(h)
            return mm_at, id_at

        # Flat emission over (group, t) slots on a virtual timeline: group
        # g+1's first steps are emitted OV steps before group g's last step,
        # so no engine idles across the group boundary. PSUM banks and sgn
        # buffers rotate on the GLOBAL step index so overlapped slots never
        # collide.
        OV = 3
        slots = []
        for g in range(NG):
            for t in range(T):
                slots.append((g * (T - OV) + t, g, t))
        slots.sort(key=lambda x: (x[0], x[1]))

        drips = [drip_schedule() if g + 1 < NG else ({}, {}) for g in range(NG)]
        pending_copies = []  # (g, ts) ev batches, emitted one slot late

        def fc2_step(g, t):
            bs = slice(col0[g], col0[g] + GROUPS[g])
            w = GROUPS[g]
            gs = g * T + t
            for h in range(HT):
                nc.tensor.matmul(
                    p2[:, gs % (2 * EVB), 0:w],
                    w2sb[:, h, :],
                    sgn1[gs % NSG][:, h, bs],
                    start=(h == 0),
                    stop=(h == HT - 1),
                    skip_group_check=True,
                )

        def ev_out(g, ts):
            bs = slice(col0[g], col0[g] + GROUPS[g])
            w = GROUPS[g]
            nb = len(ts)
            j0 = (g * T + ts[0]) % (2 * EVB)
            # copy PSUM banks j0..j0+nb-1 (mod 6) into ev[0:nb]; the bank
            # window can wrap, needing two activations
            n1 = min(nb, 2 * EVB - j0)
            nc.scalar.activation(
                ev[:, 0:n1, 0:w],
                p2[:, j0 : j0 + n1, 0:w],
                AF.Identity,
                bias=zero[0:MW],
            )
            if n1 < nb:
                nc.scalar.activation(
                    ev[:, n1:nb, 0:w],
                    p2[:, 0 : nb - n1, 0:w],
                    AF.Identity,
                    bias=zero[0:MW],
                )
            for i, tt in enumerate(ts):
                nc.sync.dma_start(evT_d[tt, :, bs], ev[:, i, 0:w])

        for _, g, t in slots:
            bs = slice(col0[g], col0[g] + GROUPS[g])
            w = GROUPS[g]
            gs = g * T + t
            # DVE: membrane update (ping-pong buffers, no WAR with sign).
            # Group 0's first update is split by h-halves so it can start
            # while the head fc1 is still producing later h-tiles.
            if t == 1 and g == 0:
                for hh in range(2):
                    hs = slice(4 * hh, 4 * hh + 4)
                    nc.vector._custom_dve(
                        MEMUPD,
                        out=mem[1][:, hs, bs],
                        in0=cur1[:, hs, bs],
                        in1=cur1[:, hs, bs],
                        s0=BETA,
                        s1=THR,
                    )
            elif t >= 1:
                src = cur1 if t == 1 else mem[(t - 1) % 2]
                nc.vector._custom_dve(
                    MEMUPD,
                    out=mem[t % 2][:, :, bs],
                    in0=src[:, :, bs],
                    in1=cur1[:, :, bs],
                    s0=BETA,
                    s1=THR,
                )
            # ACT: spike sign (group 0's t=0 split by halves to start earlier)
            msrc = cur1 if t == 0 else mem[t % 2]
            if t == 0 and g == 0:
                for hh in range(2):
                    hs = slice(4 * hh, 4 * hh + 4)
                    nc.scalar.activation(
                        sgn1[gs % NSG][:, hs, bs],
                        msrc[:, hs, bs],
                        AF.Sign,
                        bias=negone[:],
                    )
            elif g == NG - 1 and t == T - 1:
                # tail: split the last sign by halves so fc2 can overlap
                for hh in range(2):
                    hs = slice(4 * hh, 4 * hh + 4)
                    nc.scalar.activation(
                        sgn1[gs % NSG][:, hs, bs],
                        msrc[:, hs, bs],
                        AF.Sign,
                        bias=negone[:],
                    )
            else:
                nc.scalar.activation(
                    sgn1[gs % NSG][:, :, bs],
                    msrc[:, :, bs],
                    AF.Sign,
                    bias=negone[:],
                )
            # PE: drip next group's fc1 (before fc2 so it runs while fc2
            # waits on this step's sign). Identities are emitted before new
            # MMs: fc1(h+2) reuses p1[h%2], so ident(h) must read it first.
            mm_at, id_at = drips[g]
            for h in id_at.get(t, []):
                fc1_ident(g + 1, h)
            for h, k in mm_at.get(t, []):
                fc1_mm(g + 1, h, k)
            # PE: fc2 for this step
            fc2_step(g, t)
            # PE warm-keeper: in windows with no fc1 drip the PE would idle
            # between fc2 bursts and HAM-throttle to 1.2GHz; two redundant
            # matmuls per step keep the activity window busy.
            if g == NG - 1 and 1 <= t < T - 1:
                for f in range(2):
                    nc.tensor.matmul(
                        p1[0][0:MW, 0:w],
                        w2sb[:, f, :],
                        sgn1[gs % NSG][:, f, bs],
                        start=True,
                        stop=True,
                        skip_group_check=True,
                    )
            # ACT: ev copy batches, one slot after their last fc2
            for pg, pts in pending_copies:
                ev_out(pg, pts)
            pending_copies.clear()
            if t % EVB == EVB - 1:
                pending_copies.append((g, list(range(t - EVB + 1, t + 1))))
            elif t == T - 1:
                pending_copies.append((g, list(range(T - T % EVB, T))))
        for pg, pts in pending_copies:
            ev_out(pg, pts)

    nc.finalize()
    return nc


def _prep_shared(W1, b1, W2, b2):
    bf = ml_dtypes.bfloat16
    w1T = np.zeros((KT * 128, D_H_PAD), np.float32)
    w1T[:D_IN, :D_H] = W1.T
    w1T = np.ascontiguousarray(w1T.reshape(KT, 128, D_H_PAD))

    b1pad = np.zeros(D_H_PAD, np.float32)
    b1pad[:D_H] = b1
    b1r = np.ascontiguousarray(b1pad.reshape(HT, 128).T)

    w2pad = np.zeros((D_OUT, D_H_PAD), np.float32)
    w2pad[:, :D_H] = W2
    terms = []
    r = w2pad.copy()
    for _ in range(NTERMS):
        tb = r.astype(bf)
        terms.append(tb)
        r = r - tb.astype(np.float32)

    w2p = np.zeros((128, HT, MW), bf)
    for h in range(HT):
        for ti, tb in enumerate(terms):
            half = (0.5 * tb[:, 128 * h : 128 * (h + 1)].astype(np.float32)).astype(bf)
            w2p[:, h, D_OUT * ti : D_OUT * (ti + 1)] = half.T

    # spk = (sgn+1)/2 so W2@spk = 0.5*W2@sgn + 0.5*sum(W2); fold shift into v.
    v = (b2.astype(np.float64) + 0.5 * w2pad.astype(np.float64).sum(axis=1)).astype(
        np.float32
    )
    return w1T, b1r, w2p, v


def kernel(**inputs):
    global LAST_EXEC_NS
    x = np.ascontiguousarray(np.asarray(inputs["x"], dtype=np.float32))
    W1 = np.asarray(inputs["W1"], dtype=np.float32)
    b1 = np.asarray(inputs["b1"], dtype=np.float32)
    W2 = np.asarray(inputs["W2"], dtype=np.float32)
    b2 = np.asarray(inputs["b2"], dtype=np.float32)

    if "nc" not in _CACHE:
        _CACHE["nc"] = _build()
    nc = _CACHE["nc"]

    w1T, b1r, w2p, v = _prep_shared(W1, b1, W2, b2)

    in_maps = []
    for c in range(N_CORES):
        xc = x[c * BC : (c + 1) * BC]  # [BC, 784]
        xT = np.zeros((KT * 128, BC), np.float32)
        xT[:D_IN] = xc.T
        in_maps.append(
            {
                "xT": np.ascontiguousarray(xT.reshape(KT, 128, BC)),
                "w1T": w1T,
                "b1r": b1r,
                "w2p": w2p,
            }
        )

    if TRACE:
        _install_ntff_hook()
    br = run_bass_kernel_spmd(nc, in_maps, list(range(N_CORES)), trace=TRACE)
    LAST_EXEC_NS = br.exec_time_ns

    # cur2 = 0.5*W2@sgn + v, summed over the 3 bf16 terms on host.
    X = np.empty((T, B, D_OUT), np.float32)
    for c in range(N_CORES):
        evT = br.results[c]["evT"]  # [T, MW, BC]
        s = (evT[:, 0:D_OUT, :] + evT[:, D_OUT : 2 * D_OUT, :]) + evT[
            :, 2 * D_OUT : 3 * D_OUT, :
        ]
        X[:, c * BC : (c + 1) * BC, :] = np.transpose(s, (0, 2, 1)) + v

    mem2_rec = np.empty((T, B, D_OUT), np.float32)
    beta = np.float32(BETA)
    mem = X[0]
    mem2_rec[0] = mem
    for t in range(1, T):
        spk = (mem > THR).astype(np.float32)
        mem = beta * mem + X[t] - spk
        mem2_rec[t] = mem
    spk2_rec = (mem2_rec > THR).astype(np.float32)
    return spk2_rec, mem2_rec


# revision 19
# speedup vs baseline: 1.0765x; 1.0379x over previous
import re
import sys
from contextlib import ExitStack

import numpy as np

try:
    import concourse  # noqa
except ImportError:
    sys.path.insert(0, "/opt/trn_rl_repo")

import ml_dtypes
import concourse.bass as bass  # noqa
import concourse.dve_ops as dve_ops
import concourse.tile as tile
from concourse import mybir
from concourse.bass_utils import run_bass_kernel_spmd
from concourse.dve_ops import DveOp
from concourse.dve_spec import C0, C1, Spec, Src0, Src1
from concourse.dve_table_gen import dve_ver_for
from concourse.bacc import Bacc

N_CORES = 8
B = 8192
BC = B // N_CORES  # 1024 batch per core
D_IN = 784
KT = 7  # 784 -> 7 k-tiles of 128
D_IN_PAD = KT * 128  # 896
D_H = 1000
HT = 8  # 1000 -> 8 h-tiles of 128
D_H_PAD = HT * 128  # 1024
D_OUT = 10
T = 25
BETA = 0.95
THR = 1.0
NTERMS = 3  # exact bf16 decomposition of W2
MW = 30  # packed fc2 stationary width: term ti at columns 10*ti..10*ti+9
GROUPS = [512, 512]  # batch column groups; group g+1's fc1 drips through group g's loop
EVB = 3  # ev copies batched over EVB timesteps (PSUM rotation depth 2*EVB banks)
PACE_END = 17  # drip of next group's fc1 spread over steps 0..PACE_END-1
NSG = 3  # sgn buffer rotation depth
WMAX = 512

LAST_EXEC_NS = None
TRACE = False

_CACHE = {}


def _install_ntff_hook():
    try:
        import antenv.axon_hooks  # noqa

        return
    except ImportError:
        pass
    try:
        import types

        import antenv

        mod = types.ModuleType("antenv.axon_hooks")
        mod._hook = None

        def set_axon_ntff_profile_hook(h):
            mod._hook = h

        def get_axon_ntff_profile_hook():
            return mod._hook

        mod.set_axon_ntff_profile_hook = set_axon_ntff_profile_hook
        mod.get_axon_ntff_profile_hook = get_axon_ntff_profile_hook
        sys.modules["antenv.axon_hooks"] = mod
        antenv.axon_hooks = mod
        try:
            from trn_agent_boot.trn_boot import _ntff_profile_via_ctypes

            hook = _ntff_profile_via_ctypes("/opt/axon/libaxon_pjrt.so")
            if hook is not None:
                mod._hook = hook
        except Exception:
            pass
    except Exception:
        pass


def _register_memupd():
    for op in dve_ops.OPS:
        if op.name == "SNN_MEMUPD":
            return op
    spec = Spec(
        body=Src0 * C0 + Src1 - (Src0 > C1),
        reference=lambda in0, in1, s0, s1, imm2: in0 * s0
        + in1
        - (in0 > s1).astype(np.float32),
    )
    op = DveOp("SNN_MEMUPD", spec, subdim=False, uops_sha={})
    dve_ops.OPS.append(op)
    dve_ops.CUSTOM_DVE_SPECS[op.name] = op.spec
    dve_ops._SUB_OPCODE_FOR_NAME[op.name] = (
        dve_ops._CUSTOM_DVE_ROW_BASE + len(dve_ops.OPS) - 1
    )
    ver = dve_ver_for("TRN2")
    try:
        op.compile(ver)
    except ValueError as e:
        m = re.search(r'uops_sha\["%s"\]="([0-9a-f]+)"' % ver, str(e))
        if not m:
            raise
        op.uops_sha[ver] = m.group(1)
        op.compile(ver)
    return op


def _build():
    MEMUPD = _register_memupd()
    nc = Bacc()
    f32 = mybir.dt.float32
    bf16 = mybir.dt.bfloat16
    AF = mybir.ActivationFunctionType

    xT_d = nc.declare_dram_parameter("xT", [KT, 128, BC], f32, isOutput=False)
    w1T_d = nc.declare_dram_parameter("w1T", [KT, 128, D_H_PAD], f32, isOutput=False)
    b1r_d = nc.declare_dram_parameter("b1r", [128, HT], f32, isOutput=False)
    w2p_d = nc.declare_dram_parameter("w2p", [128, HT, MW], bf16, isOutput=False)
    evT_d = nc.declare_dram_parameter("evT", [T, MW, BC], f32, isOutput=True)

    col0 = [sum(GROUPS[:g]) for g in range(len(GROUPS))]  # start col of each group
    NG = len(GROUPS)

    with tile.TileContext(nc) as tc, ExitStack() as ctx:
        pool = ctx.enter_context(tc.tile_pool(name="sb", bufs=1))
        ppool = ctx.enter_context(tc.tile_pool(name="ps", bufs=1, space="PSUM"))

        xsb = pool.tile([128, KT, BC], f32)
        w1sb = pool.tile([128, KT, D_H_PAD], f32)
        cur1 = pool.tile([128, HT, BC], f32)
        mem = [pool.tile([128, HT, BC], f32, name=f"mem_{i}") for i in range(2)]
        sgn1 = [pool.tile([128, HT, BC], bf16, name=f"sgn1_{i}") for i in range(NSG)]
        w2sb = pool.tile([128, HT, MW], bf16)
        b1sb = pool.tile([128, HT], f32)
        negone = pool.tile([128, 1], f32)
        zero = pool.tile([128, 1], f32)
        ev = pool.tile([MW, EVB, WMAX], f32)

        p1 = [ppool.tile([128, WMAX], f32, name=f"p1_{i}") for i in range(2)]
        p2 = ppool.tile([MW, 2 * EVB, WMAX], f32)

        nc.gpsimd.memset(negone[:], -1.0)
        nc.gpsimd.memset(zero[:], 0.0)

        # DMA order tuned so PE can start group 0's fc1 early.
        nc.sync.dma_start(b1sb[:], b1r_d[:])
        nc.sync.dma_start(w2sb[:], w2p_d[:])
        g0 = slice(col0[0], col0[0] + GROUPS[0])
        for k in range(KT):
            nc.sync.dma_start(xsb[:, k, g0], xT_d[k, :, g0])
            nc.sync.dma_start(w1sb[:, k, 0:128], w1T_d[k, :, 0:128])
        for h in range(1, HT):
            for k in range(KT):
                nc.sync.dma_start(
                    w1sb[:, k, 128 * h : 128 * (h + 1)],
                    w1T_d[k, :, 128 * h : 128 * (h + 1)],
                )
        for g in range(1, NG):
            gs = slice(col0[g], col0[g] + GROUPS[g])
            for k in range(KT):
                nc.sync.dma_start(xsb[:, k, gs], xT_d[k, :, gs])

        def fc1_mm(g, h, k):
            bs = slice(col0[g], col0[g] + GROUPS[g])
            w = GROUPS[g]
            nc.tensor.matmul(
                p1[h % 2][:, 0:w],
                w1sb[:, k, 128 * h : 128 * (h + 1)],
                xsb[:, k, bs],
                start=(k == 0),
                stop=(k == KT - 1),
                skip_group_check=True,
            )

        def fc1_ident(g, h):
            bs = slice(col0[g], col0[g] + GROUPS[g])
            w = GROUPS[g]
            nc.scalar.activation(
                cur1[:, h, bs], p1[h % 2][:, 0:w], AF.Identity, bias=b1sb[:, h : h + 1]
            )

        # group 0's fc1 up front (the head)
        for h in range(HT):
            for k in range(KT):
                fc1_mm(0, h, k)
            fc1_ident(0, h)

        def drip_schedule():
            # spread next group's fc1 MMs over steps 0..PACE_END-1; identities
            # trail their h-tile by 2 steps so ACT never blocks on the PE.
            units = [(h, k) for h in range(HT) for k in range(KT)]
            U = len(units)
            mm_at = {}
            id_at = {}
            for u, (h, k) in enumerate(units):
                s = u * PACE_END // U
                mm_at.setdefault(s, []).append((h, k))
                if k == KT - 1:
                    id_at.setdefault(s + 2, []).append(h)
            return mm_at, id_at

        # Flat emission over (group, t) slots on a virtual timeline: group
        # g+1's first steps are emitted OV steps before group g's last step,
        # so no engine idles across the group boundary. PSUM banks and sgn
        # buffers rotate on the GLOBAL step index so overlapped slots never
        # collide.
        OV = 3
        slots = []
        for g in range(NG):
            for t in range(T):
                slots.append((g * (T - OV) + t, g, t))
        slots.sort(key=lambda x: (x[0], x[1]))

        drips = [drip_schedule() if g + 1 < NG else ({}, {}) for g in range(NG)]
        pending_copies = []  # (g, ts) ev batches, emitted one slot late

        def fc2_step(g, t):
            bs = slice(col0[g], col0[g] + GROUPS[g])
            w = GROUPS[g]
            gs = g * T + t
            for h in range(HT):
                nc.tensor.matmul(
                    p2[:, gs % (2 * EVB), 0:w],
                    w2sb[:, h, :],
                    sgn1[gs % NSG][:, h, bs],
                    start=(h == 0),
                    stop=(h == HT - 1),
                    skip_group_check=True,
                )

        def ev_out(g, ts):
            bs = slice(col0[g], col0[g] + GROUPS[g])
            w = GROUPS[g]
            nb = len(ts)
            j0 = (g * T + ts[0]) % (2 * EVB)
            # copy PSUM banks j0..j0+nb-1 (mod 6) into ev[0:nb]; the bank
            # window can wrap, needing two activations
            n1 = min(nb, 2 * EVB - j0)
            nc.scalar.activation(
                ev[:, 0:n1, 0:w],
                p2[:, j0 : j0 + n1, 0:w],
                AF.Identity,
                bias=zero[0:MW],
            )
            if n1 < nb:
                nc.scalar.activation(
                    ev[:, n1:nb, 0:w],
                    p2[:, 0 : nb - n1, 0:w],
                    AF.Identity,
                    bias=zero[0:MW],
                )
            for i, tt in enumerate(ts):
                nc.sync.dma_start(evT_d[tt, :, bs], ev[:, i, 0:w])

        TJ = NSG - 1  # group-0 staircase depth: limited by free sgn slots: h0-3 runs t<=TJ while head fc1 finishes h4-7

        def memupd_half(g, t, hs):
            bs = slice(col0[g], col0[g] + GROUPS[g])
            src = cur1 if t == 1 else mem[(t - 1) % 2]
            nc.vector._custom_dve(
                MEMUPD,
                out=mem[t % 2][:, hs, bs],
                in0=src[:, hs, bs],
                in1=cur1[:, hs, bs],
                s0=BETA,
                s1=THR,
            )

        def sign_half(g, t, hs):
            bs = slice(col0[g], col0[g] + GROUPS[g])
            gs = g * T + t
            msrc = cur1 if t == 0 else mem[t % 2]
            nc.scalar.activation(
                sgn1[gs % NSG][:, hs, bs],
                msrc[:, hs, bs],
                AF.Sign,
                bias=negone[:],
            )

        H03 = slice(0, 4)
        H47 = slice(4, 8)

        for _, g, t in slots:
            bs = slice(col0[g], col0[g] + GROUPS[g])
            w = GROUPS[g]
            gs = g * T + t
            if g == 0 and t <= TJ:
                # staircase: only the h0-3 half runs; h4-7 catches up at
                # t=TJ+1. No fc2 yet (it needs both halves), so no sgn-WAR
                # pressure while the NSG=3 slots last.
                if t >= 1:
                    memupd_half(g, t, H03)
                sign_half(g, t, H03)
                mm_at, id_at = drips[g]
                for h in id_at.get(t, []):
                    fc1_ident(g + 1, h)
                for h, k in mm_at.get(t, []):
                    fc1_mm(g + 1, h, k)
                continue
            if g == 0 and t == TJ + 1:
                # h4-7 catch-up: cur1 h4-7 just landed; run its chain plus
                # the deferred fc2/copies for t=0..TJ
                for tt in range(TJ + 1):
                    if tt >= 1:
                        memupd_half(g, tt, H47)
                    sign_half(g, tt, H47)
                    fc2_step(g, tt)
                    for pg, pts in pending_copies:
                        ev_out(pg, pts)
                    pending_copies.clear()
                    if tt % EVB == EVB - 1:
                        pending_copies.append(
                            (g, list(range(tt - EVB + 1, tt + 1)))
                        )
            # DVE: membrane update (ping-pong buffers, no WAR with sign)
            if t >= 1:
                src = cur1 if t == 1 else mem[(t - 1) % 2]
                nc.vector._custom_dve(
                    MEMUPD,
                    out=mem[t % 2][:, :, bs],
                    in0=src[:, :, bs],
                    in1=cur1[:, :, bs],
                    s0=BETA,
                    s1=THR,
                )
            # ACT: spike sign
            msrc = cur1 if t == 0 else mem[t % 2]
            if g == NG - 1 and t == T - 1:
                # tail: split the last sign by halves so fc2 can overlap
                for hh in range(2):
                    hs = slice(4 * hh, 4 * hh + 4)
                    nc.scalar.activation(
                        sgn1[gs % NSG][:, hs, bs],
                        msrc[:, hs, bs],
                        AF.Sign,
                        bias=negone[:],
                    )
            else:
                nc.scalar.activation(
                    sgn1[gs % NSG][:, :, bs],
                    msrc[:, :, bs],
                    AF.Sign,
                    bias=negone[:],
                )
            # PE: drip next group's fc1 (before fc2 so it runs while fc2
            # waits on this step's sign). Identities are emitted before new
            # MMs: fc1(h+2) reuses p1[h%2], so ident(h) must read it first.
            mm_at, id_at = drips[g]
            for h in id_at.get(t, []):
                fc1_ident(g + 1, h)
            for h, k in mm_at.get(t, []):
                fc1_mm(g + 1, h, k)
            # PE: fc2 for this step
            fc2_step(g, t)
            # PE warm-keeper: in windows with no fc1 drip the PE would idle
            # between fc2 bursts and HAM-throttle to 1.2GHz; two redundant
            # matmuls per step keep the activity window busy.
            if g == NG - 1 and 1 <= t < T - 1:
                for f in range(2):
                    nc.tensor.matmul(
                        p1[0][0:MW, 0:w],
                        w2sb[:, f, :],
                        sgn1[gs % NSG][:, f, bs],
                        start=True,
                        stop=True,
                        skip_group_check=True,
                    )
            # ACT: ev copy batches, one slot after their last fc2
            for pg, pts in pending_copies:
                ev_out(pg, pts)
            pending_copies.clear()
            if t % EVB == EVB - 1:
                pending_copies.append((g, list(range(t - EVB + 1, t + 1))))
            elif t == T - 1:
                pending_copies.append((g, list(range(T - T % EVB, T))))
        for pg, pts in pending_copies:
            ev_out(pg, pts)

    nc.finalize()
    return nc


def _prep_shared(W1, b1, W2, b2):
    bf = ml_dtypes.bfloat16
    w1T = np.zeros((KT * 128, D_H_PAD), np.float32)
    w1T[:D_IN, :D_H] = W1.T
    w1T = np.ascontiguousarray(w1T.reshape(KT, 128, D_H_PAD))

    b1pad = np.zeros(D_H_PAD, np.float32)
    b1pad[:D_H] = b1
    b1r = np.ascontiguousarray(b1pad.reshape(HT, 128).T)

    w2pad = np.zeros((D_OUT, D_H_PAD), np.float32)
    w2pad[:, :D_H] = W2
    terms = []
    r = w2pad.copy()
    for _ in range(NTERMS):
        tb = r.astype(bf)
        terms.append(tb)
        r = r - tb.astype(np.float32)

    w2p = np.zeros((128, HT, MW), bf)
    for h in range(HT):
        for ti, tb in enumerate(terms):
            half = (0.5 * tb[:, 128 * h : 128 * (h + 1)].astype(np.float32)).astype(bf)
            w2p[:, h, D_OUT * ti : D_OUT * (ti + 1)] = half.T

    # spk = (sgn+1)/2 so W2@spk = 0.5*W2@sgn + 0.5*sum(W2); fold shift into v.
    v = (b2.astype(np.float64) + 0.5 * w2pad.astype(np.float64).sum(axis=1)).astype(
        np.float32
    )
    return w1T, b1r, w2p, v


def kernel(**inputs):
    global LAST_EXEC_NS
    x = np.ascontiguousarray(np.asarray(inputs["x"], dtype=np.float32))
    W1 = np.asarray(inputs["W1"], dtype=np.float32)
    b1 = np.asarray(inputs["b1"], dtype=np.float32)
    W2 = np.asarray(inputs["W2"], dtype=np.float32)
    b2 = np.asarray(inputs["b2"], dtype=np.float32)

    if "nc" not in _CACHE:
        _CACHE["nc"] = _build()
    nc = _CACHE["nc"]

    w1T, b1r, w2p, v = _prep_shared(W1, b1, W2, b2)

    in_maps = []
    for c in range(N_CORES):
        xc = x[c * BC : (c + 1) * BC]  # [BC, 784]
        xT = np.zeros((KT * 128, BC), np.float32)
        xT[:D_IN] = xc.T
        in_maps.append(
            {
                "xT": np.ascontiguousarray(xT.reshape(KT, 128, BC)),
                "w1T": w1T,
                "b1r": b1r,
                "w2p": w2p,
            }
        )

    if TRACE:
        _install_ntff_hook()
    try:
        br = run_bass_kernel_spmd(nc, in_maps, list(range(N_CORES)), trace=TRACE)
    except Exception:
        # transient NRT_EXEC_UNIT_UNRECOVERABLE failures have been observed on
        # first runs of a freshly compiled NEFF; one retry has always recovered
        br = run_bass_kernel_spmd(nc, in_maps, list(range(N_CORES)), trace=TRACE)
    LAST_EXEC_NS = br.exec_time_ns

    # cur2 = 0.5*W2@sgn + v, summed over the 3 bf16 terms on host.
    X = np.empty((T, B, D_OUT), np.float32)
    for c in range(N_CORES):
        evT = br.results[c]["evT"]  # [T, MW, BC]
        s = (evT[:, 0:D_OUT, :] + evT[:, D_OUT : 2 * D_OUT, :]) + evT[
            :, 2 * D_OUT : 3 * D_OUT, :
        ]
        X[:, c * BC : (c + 1) * BC, :] = np.transpose(s, (0, 2, 1)) + v

    mem2_rec = np.empty((T, B, D_OUT), np.float32)
    beta = np.float32(BETA)
    mem = X[0]
    mem2_rec[0] = mem
    for t in range(1, T):
        spk = (mem > THR).astype(np.float32)
        mem = beta * mem + X[t] - spk
        mem2_rec[t] = mem
    spk2_rec = (mem2_rec > THR).astype(np.float32)
    return spk2_rec, mem2_rec
